# revision 13
# baseline (speedup 1.0000x reference)
"""Trainium2 Bass kernel for the unsupervised-entropy loss.

intra = mean_r H_r where H_r = entropy(softmax(-d2(x_r, m))).
Softmax is shift-invariant, so with unit-norm m rows the logits reduce to
z = 2 x m^T (the ||x||^2 and ||m||^2 terms drop).  Per row:
  S = sum_j exp(z_j),  W = sum_j z_j exp(z_j),  H = log S - W/S
(invariant to any constant logit shift, so no max-subtraction needed; z is
bounded by ~|2 x.m| <= ~13 which exp handles comfortably in fp32).

The final answer only needs sums over rows, so any row permutation is fine.
We exploit that to give the DMA large contiguous descriptors: partition p of
chunk c holds rows base_c + p*CG + g (g = 0..CG-1), i.e. each partition reads
one contiguous run per chunk (f32->bf16 cast on load via SWDGE).  Chunks are
2 MiB except the last two (1 MiB) so the compute trailing the final DMA is
short.  All constants ship in ONE DMA.  The x chunk loads bypass the tile
dependency tracker entirely: they land in a raw 3-slot SBUF ring and are
synchronized with two explicit semaphores (xSem: DMA completions, +16 per
chunk; warSem: +1 per xT eviction, gating slot reuse).  Tile's automatic
DMA wait assignment is conservative -- consumers ended up waiting on the
*latest* issued chunk, keeping the pipeline ~2 chunks behind the DMA
stream; explicit thresholds make every transpose wait on exactly its own
chunk.

Per 1024-row half-block i the stages are
  T(i):   8 PE transposes (bf16, data stationary)   -> psT [128, 8, 128]
  EV(i):  one DVE copy (2x perf mode)               -> xT [128, 1024] SBUF
  Z(i):   2 bf16 matmuls, shared 64-col weight 2*m^T, col-tiled: chunk A
          -> psZ[0:64], chunk B -> psZ[64:128] (one bank)
  EXP(i): ACT exp(psZ) -> E bf16
  STT(i): DVE z*E -> P bf16
  R(i):   2 reduce matmuls, zero-padded indicators, col-tiled:
          S -> psSW[0:4], W -> psSW[32:36] (one bank)
  SC(i):  one ACT copy psSW[0:36] -> stats[36, 32, 512]

Engines have in-order queues, so the loop is explicitly software-pipelined:
iteration t issues T(t), EV(t-1), Z(t-2), EXP(t-3), STT(t-4), R(t-5),
SC(t-6) — every issued op's producers completed in earlier periods.  A
burst of dummy ident matmuls runs during the first chunk's DMA window to
flip the PE's HAM clock gate to 2.4 GHz before real work arrives.

Final reduction is split in halves; the first half (stats blocks 0:16) is
folded into the pipeline shadow around iterations 24-30.  Host reduces the
[128,2] per-partition partial sums and adds the (tiny) inter term.
"""

import json

import numpy as np
import ml_dtypes

import concourse.bass as _bass
import concourse.tile as _tile
from concourse import mybir
from concourse.bass_utils import run_bass_kernel_spmd
from concourse.vector_clock import ScopedClock

F32 = mybir.dt.float32
BF16 = mybir.dt.bfloat16
N, D, K = 262144, 128, 64
NCORES = 8
NSHARD = N // NCORES          # 32768 rows per core
NBIG = 6                      # 2 MiB chunks (CG=32)
NSMALL = 4                    # 1 MiB chunks (CG=16): 1 leading + 3 trailing
BIGROWS = 128 * 32
SMALLROWS = 128 * 16
NB = NBIG * 4 + NSMALL * 2    # 32 half-blocks of 1024 rows
EPS = 1e-16
LAMB = 1.0
NWARM = 44                    # HAM warm-up matmuls


# ---- workarounds: this walrus build rejects >1 sync wait per instruction ----

def _split_multiwait(json_bytes: bytes) -> bytes:
    data = json.loads(json_bytes)
    counter = [0]
    for fn in data["functions"]:
        for blk in fn["blocks"]:
            new_insts = []
            for inst in blk["instructions"]:
                si = inst.get("sync_info")
                waits = (si or {}).get("on_wait") or []
                if len(waits) > 1:
                    for w in waits[:-1]:
                        counter[0] += 1
                        new_insts.append({
                            "debug": inst.get("debug"),
                            "engine": inst["engine"],
                            "ins": [],
                            "name": f"splitw_{counter[0]}_{inst['name']}",
                            "opcode": "EventSemaphore",
                            "outs": [],
                            "sync_info": {"on_update": [], "on_wait": [w]},
                        })
                    si["on_wait"] = [waits[-1]]
                new_insts.append(inst)
            blk["instructions"] = new_insts
    return json.dumps(data).encode()


class PatchedBass(_bass.Bass):
    def to_json_bytes(self) -> bytes:
        return _split_multiwait(super().to_json_bytes())


class SplitDrainTileContext(_tile.TileContext):
    def _drain_and_barrier(self, tick_clock, wait_clock):
        drain_inst = self.nc.sync.drain()
        wait_clock.add_sem_waits(
            drain_inst.ins, ScopedClock({None: tick_clock.global_clock})
        )
        si = drain_inst.ins.sync_info
        if si is not None and len(si.on_wait) > 1:
            waits = list(si.on_wait)
            si.on_wait = waits[:1]
            drain_inst.ins.sync_info = si
            for w in waits[1:]:
                d2 = self.nc.sync.drain()
                si2 = d2.ins.sync_info
                if si2 is None:
                    import copy
                    si2 = copy.copy(si)
                si2.on_wait = [w]
                si2.on_update = []
                d2.ins.sync_info = si2
        self.nc.all_engine_barrier()
        assert self.sems is not None
        popped = self.nc._tile_sem_poison_stack.pop()
        assert popped is self._sem_poison
        self.nc.clear_and_free_semaphores(list(self.sems.allocated().values()))
        self.nc.all_engine_barrier()


# ------------------------------ kernel build ------------------------------

_CACHE = {}


def _build():
    if "nc" in _CACHE:
        return _CACHE["nc"]
    nc = PatchedBass("TRN2", target_bir_lowering=False, debug=False)
    xs_ap = nc.dram_tensor("xs", [NSHARD, D], F32, kind="ExternalInput").ap()
    consts_ap = nc.dram_tensor("consts", [128, 200], BF16,
                               kind="ExternalInput").ap()
    out_ap = nc.dram_tensor("out", [128, 2], F32, kind="ExternalOutput").ap()

    Exp = mybir.ActivationFunctionType.Exp
    Ln = mybir.ActivationFunctionType.Ln
    MUL = mybir.AluOpType.mult
    ADD = mybir.AluOpType.add

    bigrows = NBIG * BIGROWS
    xs_front = xs_ap[0:SMALLROWS].rearrange("(p g) d -> p (g d)",
                                            p=128, g=16)
    xs_big = xs_ap[SMALLROWS:SMALLROWS + bigrows].rearrange(
        "(c p g) d -> c p (g d)", p=128, g=32)
    xs_back = xs_ap[SMALLROWS + bigrows:NSHARD].rearrange(
        "(c p g) d -> c p (g d)", p=128, g=16)

    NCHUNK = NBIG + NSMALL    # chunk 0 small, 1..NBIG big, rest small

    def chunk_cols(j):
        return 32 * D if 1 <= j <= NBIG else 16 * D

    # half-block index -> (chunk index, half within chunk)
    def locate(b):
        if b < 2:
            return 0, b
        if b < 2 + NBIG * 4:
            return 1 + (b - 2) // 4, (b - 2) % 4
        bb = b - 2 - NBIG * 4
        return 1 + NBIG + bb // 2, bb % 2

    from contextlib import ExitStack
    octx = ExitStack()
    # one raw SBUF buffer per chunk: tile's per-tensor dependency tracking
    # then gives every transpose exactly one DMA dep (its own chunk)
    xbuf = []
    for j in range(NBIG + NSMALL):
        xbuf.append(octx.enter_context(
            nc.sbuf_tensor(f"xbuf{j}", [128, chunk_cols(j)], BF16)))

    with SplitDrainTileContext(nc) as tc:
        with tc.tile_pool(name="const", bufs=1) as const, \
             tc.tile_pool(name="xtp", bufs=3) as xtp, \
             tc.tile_pool(name="ep", bufs=3) as ep, \
             tc.tile_pool(name="stage", bufs=1) as stage, \
             tc.tile_pool(name="fin", bufs=1) as fin, \
             tc.tile_pool(name="psT", bufs=2, space="PSUM") as psTp, \
             tc.tile_pool(name="psZ", bufs=3, space="PSUM") as psZp, \
             tc.tile_pool(name="psSW", bufs=2, space="PSUM") as psSWp:

            consts = const.tile([128, 200], BF16)
            nc.sync.dma_start(out=consts[:], in_=consts_ap[:])
            pad64 = consts[:, 0:64]
            indS = consts[:, 64:68]
            indW = consts[:, 68:72]
            ident = consts[:, 72:200]

            stats = stage.tile([36, NB, 512], F32)

            # issue ALL chunk loads up front; buffers are exclusive per
            # chunk so the SWDGE queue streams them back to back
            for j in range(NCHUNK):
                slot = xbuf[j].ap()
                if j == 0:
                    nc.gpsimd.dma_start(out=slot[:], in_=xs_front[:])
                elif j <= NBIG:
                    nc.gpsimd.dma_start(out=slot[:], in_=xs_big[j - 1])
                else:
                    nc.gpsimd.dma_start(out=slot[:],
                                        in_=xs_back[j - 1 - NBIG])

            # HAM warm-up: dummy matmuls while the first chunk loads
            warm = psSWp.tile([128, 128], F32, name="warm", tag="warm",
                              bufs=1)
            for _ in range(NWARM):
                nc.tensor.matmul(warm[:], ident, ident, start=True, stop=True)

            # final-stage tiles (ranges A=[0:16], C=[16:24] folded into
            # the loop; D=[24:32] runs in the epilogue)
            s128a = fin.tile([128, 128], F32)
            w128a = fin.tile([128, 128], F32)
            lnA = fin.tile([128, 128], F32)
            lsA = fin.tile([128, 1], F32)
            rSA = fin.tile([128, 128], F32)
            scrA = fin.tile([128, 128], F32)
            wsA = fin.tile([128, 1], F32)
            s128c = fin.tile([128, 64], F32)
            w128c = fin.tile([128, 64], F32)
            lnC = fin.tile([128, 64], F32)
            lsC = fin.tile([128, 1], F32)
            rSC = fin.tile([128, 64], F32)
            scrC = fin.tile([128, 64], F32)
            wsC = fin.tile([128, 1], F32)

            psTs = {}
            xTs = {}
            psZs = {}
            Es = {}
            Ps = {}
            psSWs = {}

            for t in range(NB + 6):
                # T(t): transposes on PE, reading the raw ring
                if t < NB:
                    cj, h = locate(t)
                    slot = xbuf[cj].ap()
                    psTs[t] = psTp.tile([128, 8, 128], BF16, name="psT",
                                        tag="psT")
                    for j in range(8):
                        nc.tensor.transpose(
                            psTs[t][:, j, :],
                            slot[:, (h * 8 + j) * 128:(h * 8 + j + 1) * 128],
                            ident)

                # EV(t-1): evict xT on DVE; +1 on warSem frees ring slots
                i = t - 1
                if 0 <= i < NB:
                    xTs[i] = xtp.tile([128, 8, 128], BF16, name="xT",
                                      tag="xT")
                    nc.vector.tensor_copy(xTs[i][:], psTs[i][:])
                    del psTs[i]

                # Z(t-2): logits matmuls on PE (col-tiled halves)
                i = t - 2
                if 0 <= i < NB:
                    xTf = xTs[i][:].rearrange("p g r -> p (g r)")
                    psZs[i] = psZp.tile([128, 512], F32, name="psZ",
                                        tag="psZ")
                    nc.tensor.matmul(psZs[i][0:64, :], pad64,
                                     xTf[:, 0:512], start=True, stop=True,
                                     tile_position=(0, 0))
                    nc.tensor.matmul(psZs[i][64:128, :], pad64,
                                     xTf[:, 512:1024], start=True, stop=True,
                                     tile_position=(0, 64))
                    del xTs[i]

                # EXP(t-3) on ACT
                i = t - 3
                if 0 <= i < NB:
                    Es[i] = ep.tile([128, 512], BF16, tag="E", name="E")
                    nc.scalar.activation(Es[i][:], psZs[i][:], Exp)

                # STT(t-4) on DVE
                i = t - 4
                if 0 <= i < NB:
                    Ps[i] = ep.tile([128, 512], BF16, tag="P", name="P")
                    nc.vector.scalar_tensor_tensor(Ps[i][:], psZs[i][:], 1.0,
                                                   Es[i][:], MUL, MUL)
                    del psZs[i]

                # R(t-5): reduce matmuls on PE (col-tiled S and W)
                i = t - 5
                if 0 <= i < NB:
                    psSWs[i] = psSWp.tile([36, 512], F32, name="psSW",
                                          tag="psSW")
                    nc.tensor.matmul(psSWs[i][0:4, :], indS, Es[i][:],
                                     start=True, stop=True,
                                     tile_position=(0, 0))
                    nc.tensor.matmul(psSWs[i][32:36, :], indW, Ps[i][:],
                                     start=True, stop=True,
                                     tile_position=(0, 32))
                    del Es[i]
                    del Ps[i]

                # SC(t-6): stats eviction on ACT
                i = t - 6
                if 0 <= i < NB:
                    nc.scalar.copy(stats[:, i, :], psSWs[i][:])
                    del psSWs[i]

                # in-loop final reduction for ranges A and C, spread out
                # so no engine head-of-line blocks on a DMA completion
                if t == 22:   # SC(15) issued at t=21
                    nc.sync.dma_start(
                        out=s128a[:],
                        in_=stats[0:2, 0:16].rearrange("p a b -> p (a b)"))
                    nc.sync.dma_start(
                        out=w128a[:],
                        in_=stats[32:34, 0:16].rearrange("p a b -> p (a b)"))
                elif t == 28:
                    nc.scalar.activation(lnA[:], s128a[:], Ln,
                                         accum_out=lsA[:])
                elif t == 30:
                    nc.scalar.activation(rSA[:], lnA[:], Exp, scale=-1.0)
                elif t == 32:
                    nc.vector.scalar_tensor_tensor(scrA[:], w128a[:], 1.0,
                                                   rSA[:], MUL, MUL,
                                                   accum_out=wsA[:])
                    nc.sync.dma_start(
                        out=s128c[:],
                        in_=stats[0:2, 16:24].rearrange("p a b -> p (a b)"))
                    nc.sync.dma_start(
                        out=w128c[:],
                        in_=stats[32:34, 16:24].rearrange("p a b -> p (a b)"))
                elif t == 36:
                    nc.scalar.activation(lnC[:], s128c[:], Ln,
                                         accum_out=lsC[:])
                elif t == 37:
                    nc.scalar.activation(rSC[:], lnC[:], Exp, scale=-1.0)

            # epilogue: finish range C, then range D = blocks [24:32]
            nc.vector.scalar_tensor_tensor(scrC[:], w128c[:], 1.0, rSC[:],
                                           MUL, MUL, accum_out=wsC[:])
            s128d = fin.tile([128, 64], F32)
            w128d = fin.tile([128, 64], F32)
            nc.sync.dma_start(
                out=s128d[:],
                in_=stats[0:2, 24:32].rearrange("p a b -> p (a b)"))
            nc.sync.dma_start(
                out=w128d[:],
                in_=stats[32:34, 24:32].rearrange("p a b -> p (a b)"))

            lnD = fin.tile([128, 64], F32)
            lsD = fin.tile([128, 1], F32)
            nc.scalar.activation(lnD[:], s128d[:], Ln, accum_out=lsD[:])
            rSD = fin.tile([128, 64], F32)
            nc.scalar.activation(rSD[:], lnD[:], Exp, scale=-1.0)
            scrD = fin.tile([128, 64], F32)
            wsD = fin.tile([128, 1], F32)
            nc.vector.scalar_tensor_tensor(scrD[:], w128d[:], 1.0, rSD[:],
                                           MUL, MUL, accum_out=wsD[:])
            lsAC = fin.tile([128, 1], F32)
            wsAC = fin.tile([128, 1], F32)
            nc.vector.tensor_tensor(lsAC[:], lsA[:], lsC[:], ADD)
            nc.vector.tensor_tensor(wsAC[:], wsA[:], wsC[:], ADD)
            ob = fin.tile([128, 2], F32)
            nc.vector.tensor_tensor(ob[:, 0:1], lsAC[:], lsD[:], ADD)
            nc.vector.tensor_tensor(ob[:, 1:2], wsAC[:], wsD[:], ADD)
            nc.sync.dma_start(out=out_ap[:], in_=ob[:])

    _CACHE["nc"] = nc
    _CACHE["octx"] = octx
    return nc


def _entropy_np(p):
    p = np.where(p <= 0, EPS, p)
    p = np.where(p >= 1, 1.0 - EPS, p)
    return -np.sum(p * np.log(p), axis=-1)


def kernel(x, m):
    nc = _build()

    mt2 = (2.0 * np.float64(m).T).astype(np.float32)       # [128, 64]
    consts = np.zeros((128, 200), dtype=ml_dtypes.bfloat16)
    consts[:, 0:64] = mt2.astype(ml_dtypes.bfloat16)       # pad64
    consts[0:K, 64] = 1                                    # indS col 0
    consts[K:128, 65] = 1                                  # indS col 1
    consts[0:K, 68] = 1                                    # indW col 0
    consts[K:128, 69] = 1                                  # indW col 1
    consts[:, 72:200] = np.eye(128, dtype=ml_dtypes.bfloat16)

    in_maps = []
    for c in range(NCORES):
        in_maps.append({
            "xs": np.ascontiguousarray(x[c * NSHARD:(c + 1) * NSHARD]),
            "consts": consts,
        })
    _CACHE["last_in_maps"] = in_maps
    res = run_bass_kernel_spmd(nc, in_maps, core_ids=list(range(NCORES)))

    tot_ls = 0.0
    tot_ws = 0.0
    for c in range(NCORES):
        o = np.float64(res.results[c]["out"])
        tot_ls += o[:, 0].sum()
        tot_ws += o[:, 1].sum()
    intra = (tot_ls - tot_ws) / N

    # inter term on host (tiny), replicating the reference exactly
    m64 = np.float64(m)
    mu = m64.mean(axis=0)
    d2 = ((mu[None, :] - m64) ** 2).sum(axis=1)
    zl = -d2
    zl -= zl.max()
    e = np.exp(zl)
    p = e / e.sum()
    inter = _entropy_np(p)

    total = intra - LAMB * inter
    return (np.float32(total), np.float32(intra), np.float32(inter))


# revision 16
# speedup vs baseline: 1.0505x; 1.0505x over previous
"""Trainium2 Bass kernel for the unsupervised-entropy loss.

intra = mean_r H_r where H_r = entropy(softmax(-d2(x_r, m))).
Softmax is shift-invariant, so with unit-norm m rows the logits reduce to
z = 2 x m^T (the ||x||^2 and ||m||^2 terms drop).  Per row:
  S = sum_j exp(z_j),  W = sum_j z_j exp(z_j),  H = log S - W/S
(invariant to any constant logit shift, so no max-subtraction needed; z is
bounded by ~|2 x.m| <= ~13 which exp handles comfortably in fp32).

The final answer only needs sums over rows, so any row permutation is fine.
We exploit that to give the DMA large contiguous descriptors: partition p of
chunk c holds rows base_c + p*CG + g (g = 0..CG-1), i.e. each partition reads
one contiguous run per chunk (f32->bf16 cast on load via SWDGE).  Chunks are
2 MiB except the last two (1 MiB) so the compute trailing the final DMA is
short.  All constants ship in ONE DMA.  The x chunk loads bypass the tile
dependency tracker entirely: they land in a raw 3-slot SBUF ring and are
synchronized with two explicit semaphores (xSem: DMA completions, +16 per
chunk; warSem: +1 per xT eviction, gating slot reuse).  Tile's automatic
DMA wait assignment is conservative -- consumers ended up waiting on the
*latest* issued chunk, keeping the pipeline ~2 chunks behind the DMA
stream; explicit thresholds make every transpose wait on exactly its own
chunk.

Per 1024-row half-block i the stages are
  T(i):   8 PE transposes (bf16, data stationary)   -> psT [128, 8, 128]
  EV(i):  one DVE copy (2x perf mode)               -> xT [128, 1024] SBUF
  Z(i):   2 bf16 matmuls, shared 64-col weight 2*m^T, col-tiled: chunk A
          -> psZ[0:64], chunk B -> psZ[64:128] (one bank)
  EXP(i): ACT exp(psZ) -> E bf16
  STT(i): DVE z*E -> P bf16
  R(i):   2 reduce matmuls, zero-padded indicators, col-tiled:
          S -> psSW[0:4], W -> psSW[32:36] (one bank)
  SC(i):  one ACT copy psSW[0:36] -> stats[36, 32, 512]

Engines have in-order queues, so the loop is explicitly software-pipelined:
iteration t issues T(t), EV(t-1), Z(t-2), EXP(t-3), STT(t-4), R(t-5),
SC(t-6) — every issued op's producers completed in earlier periods.  A
burst of dummy ident matmuls runs during the first chunk's DMA window to
flip the PE's HAM clock gate to 2.4 GHz before real work arrives.

Final reduction is split in halves; the first half (stats blocks 0:16) is
folded into the pipeline shadow around iterations 24-30.  Host reduces the
[128,2] per-partition partial sums and adds the (tiny) inter term.
"""

import json

import numpy as np
import ml_dtypes

import concourse.bass as _bass
import concourse.tile as _tile
from concourse import mybir
from concourse.bass_utils import run_bass_kernel_spmd
from concourse.vector_clock import ScopedClock

F32 = mybir.dt.float32
BF16 = mybir.dt.bfloat16
N, D, K = 262144, 128, 64
NCORES = 8
NSHARD = N // NCORES          # 32768 rows per core
NBIG = 7                      # 2 MiB chunks (CG=32)
NSMALL = 2                    # 1 MiB chunks (CG=16) at the end
BIGROWS = 128 * 32
SMALLROWS = 128 * 16
NB = NBIG * 4 + NSMALL * 2    # 32 half-blocks of 1024 rows
EPS = 1e-16
LAMB = 1.0
NWARM = 44                    # HAM warm-up matmuls


# ---- workarounds: this walrus build rejects >1 sync wait per instruction ----

def _split_multiwait(json_bytes: bytes) -> bytes:
    data = json.loads(json_bytes)
    counter = [0]
    for fn in data["functions"]:
        for blk in fn["blocks"]:
            new_insts = []
            for inst in blk["instructions"]:
                si = inst.get("sync_info")
                waits = (si or {}).get("on_wait") or []
                if len(waits) > 1:
                    for w in waits[:-1]:
                        counter[0] += 1
                        new_insts.append({
                            "debug": inst.get("debug"),
                            "engine": inst["engine"],
                            "ins": [],
                            "name": f"splitw_{counter[0]}_{inst['name']}",
                            "opcode": "EventSemaphore",
                            "outs": [],
                            "sync_info": {"on_update": [], "on_wait": [w]},
                        })
                    si["on_wait"] = [waits[-1]]
                new_insts.append(inst)
            blk["instructions"] = new_insts
    return json.dumps(data).encode()


class PatchedBass(_bass.Bass):
    def to_json_bytes(self) -> bytes:
        return _split_multiwait(super().to_json_bytes())


class SplitDrainTileContext(_tile.TileContext):
    def _drain_and_barrier(self, tick_clock, wait_clock):
        drain_inst = self.nc.sync.drain()
        wait_clock.add_sem_waits(
            drain_inst.ins, ScopedClock({None: tick_clock.global_clock})
        )
        si = drain_inst.ins.sync_info
        if si is not None and len(si.on_wait) > 1:
            waits = list(si.on_wait)
            si.on_wait = waits[:1]
            drain_inst.ins.sync_info = si
            for w in waits[1:]:
                d2 = self.nc.sync.drain()
                si2 = d2.ins.sync_info
                if si2 is None:
                    import copy
                    si2 = copy.copy(si)
                si2.on_wait = [w]
                si2.on_update = []
                d2.ins.sync_info = si2
        self.nc.all_engine_barrier()
        assert self.sems is not None
        popped = self.nc._tile_sem_poison_stack.pop()
        assert popped is self._sem_poison
        self.nc.clear_and_free_semaphores(list(self.sems.allocated().values()))
        self.nc.all_engine_barrier()


# ------------------------------ kernel build ------------------------------

_CACHE = {}


def _build():
    if "nc" in _CACHE:
        return _CACHE["nc"]
    nc = PatchedBass("TRN2", target_bir_lowering=False, debug=False)
    xs_ap = nc.dram_tensor("xs", [NSHARD, D], F32, kind="ExternalInput").ap()
    consts_ap = nc.dram_tensor("consts", [128, 200], BF16,
                               kind="ExternalInput").ap()
    out_ap = nc.dram_tensor("out", [128, 2], F32, kind="ExternalOutput").ap()

    Exp = mybir.ActivationFunctionType.Exp
    Ln = mybir.ActivationFunctionType.Ln
    MUL = mybir.AluOpType.mult
    ADD = mybir.AluOpType.add

    bigrows = NBIG * BIGROWS
    xs_big = xs_ap[0:bigrows].rearrange("(c p g) d -> c p (g d)", p=128, g=32)
    xs_small = xs_ap[bigrows:NSHARD].rearrange("(c p g) d -> c p (g d)",
                                               p=128, g=16)

    NCHUNK = NBIG + NSMALL

    def chunk_cols(j):
        return 32 * D if j < NBIG else 16 * D

    # half-block index -> (chunk index, half within chunk)
    def locate(b):
        if b < NBIG * 4:
            return b // 4, b % 4
        bb = b - NBIG * 4
        return NBIG + bb // 2, bb % 2

    from contextlib import ExitStack
    octx = ExitStack()
    # one raw SBUF buffer per chunk: tile's per-tensor dependency tracking
    # then gives every transpose exactly one DMA dep (its own chunk)
    xbuf = []
    for j in range(NBIG + NSMALL):
        xbuf.append(octx.enter_context(
            nc.sbuf_tensor(f"xbuf{j}", [128, chunk_cols(j)], BF16)))

    with SplitDrainTileContext(nc) as tc:
        with tc.tile_pool(name="const", bufs=1) as const, \
             tc.tile_pool(name="xtp", bufs=3) as xtp, \
             tc.tile_pool(name="ep", bufs=3) as ep, \
             tc.tile_pool(name="stage", bufs=1) as stage, \
             tc.tile_pool(name="fin", bufs=1) as fin, \
             tc.tile_pool(name="psT", bufs=2, space="PSUM") as psTp, \
             tc.tile_pool(name="psZ", bufs=2, space="PSUM") as psZp, \
             tc.tile_pool(name="psSW", bufs=2, space="PSUM") as psSWp:

            consts = const.tile([128, 200], BF16)
            nc.sync.dma_start(out=consts[:], in_=consts_ap[:])
            pad64 = consts[:, 0:64]
            indS = consts[:, 64:68]
            indW = consts[:, 68:72]
            ident = consts[:, 72:200]

            stats = stage.tile([100, NB // 2, 512], F32)

            # issue ALL chunk loads up front; buffers are exclusive per
            # chunk so the SWDGE queue streams them back to back
            for j in range(NCHUNK):
                slot = xbuf[j].ap()
                if j < NBIG:
                    nc.gpsimd.dma_start(out=slot[:], in_=xs_big[j])
                else:
                    nc.gpsimd.dma_start(out=slot[:],
                                        in_=xs_small[j - NBIG])

            # HAM warm-up: dummy matmuls while the first chunk loads
            # (borrows a psSW ring slot; real users are far later)
            warm = psSWp.tile([100, 512], F32, name="warm", tag="psSW")
            for _ in range(NWARM):
                nc.tensor.matmul(warm[0:36, 0:128], ident[:, 0:36], ident,
                                 start=True, stop=True)

            # final-stage tiles (ranges A=[0:16], C=[16:24] folded into
            # the loop; D=[24:32] runs in the epilogue)
            s128a = fin.tile([128, 128], F32)
            w128a = fin.tile([128, 128], F32)
            lnA = fin.tile([128, 128], F32)
            lsA = fin.tile([128, 1], F32)
            rSA = fin.tile([128, 128], F32)
            scrA = fin.tile([128, 128], F32)
            wsA = fin.tile([128, 1], F32)
            s128c = fin.tile([128, 64], F32)
            w128c = fin.tile([128, 64], F32)
            lnC = fin.tile([128, 64], F32)
            lsC = fin.tile([128, 1], F32)
            rSC = fin.tile([128, 64], F32)
            scrC = fin.tile([128, 64], F32)
            wsC = fin.tile([128, 1], F32)

            psTs = {}
            xTs = {}
            psZs = {}
            Es = {}
            Ps = {}
            psSWs = {}

            for t in range(NB + 10):
                # T(t): transposes on PE, reading the raw ring
                if t < NB:
                    cj, h = locate(t)
                    slot = xbuf[cj].ap()
                    psTs[t] = psTp.tile([128, 8, 128], BF16, name="psT",
                                        tag="psT")
                    for j in range(8):
                        nc.tensor.transpose(
                            psTs[t][:, j, :],
                            slot[:, (h * 8 + j) * 128:(h * 8 + j + 1) * 128],
                            ident)

                # EV(t-1): evict xT on DVE; +1 on warSem frees ring slots
                i = t - 1
                if 0 <= i < NB:
                    xTs[i] = xtp.tile([128, 8, 128], BF16, name="xT",
                                      tag="xT")
                    nc.vector.tensor_copy(xTs[i][:], psTs[i][:])
                    del psTs[i]

                # Z(t-2): logits matmuls on PE (col-tiled halves); two
                # consecutive blocks share one [128,1024] psZ tile so the
                # exp / z*E ops amortize their fixed cost over 1024 cols
                i = t - 2
                if 0 <= i < NB:
                    q, hf = i // 2, i % 2
                    if hf == 0:
                        psZs[q] = psZp.tile([128, 1024], F32, name="psZ",
                                            tag="psZ")
                    off = 512 * hf
                    xTf = xTs[i][:].rearrange("p g r -> p (g r)")
                    nc.tensor.matmul(psZs[q][0:64, off:off + 512], pad64,
                                     xTf[:, 0:512], start=True, stop=True,
                                     tile_position=(0, 0))
                    nc.tensor.matmul(psZs[q][64:128, off:off + 512], pad64,
                                     xTf[:, 512:1024], start=True, stop=True,
                                     tile_position=(0, 64))
                    del xTs[i]

                # EXP(pair) on ACT, one op per two blocks
                i = t - 4
                if 0 <= i < NB and i % 2 == 0:
                    q = i // 2
                    Es[q] = ep.tile([128, 1024], BF16, tag="E", name="E")
                    nc.scalar.activation(Es[q][:], psZs[q][:], Exp)

                # STT(pair) on DVE
                i = t - 5
                if 0 <= i < NB and i % 2 == 0:
                    q = i // 2
                    Ps[q] = ep.tile([128, 1024], BF16, tag="P", name="P")
                    nc.vector.scalar_tensor_tensor(Ps[q][:], psZs[q][:], 1.0,
                                                   Es[q][:], MUL, MUL)
                    del psZs[q]

                # R(t-6): reduce matmuls on PE; both blocks of a pair
                # land in ONE PSUM bank via col-tiling (even block at
                # partitions 0:4/32:36, odd block at 64:68/96:100)
                i = t - 6
                if 0 <= i < NB:
                    q, off = i // 2, 512 * (i % 2)
                    if i % 2 == 0:
                        psSWs[q] = psSWp.tile([100, 512], F32, name="psSW",
                                              tag="psSW")
                    pb = 64 * (i % 2)
                    nc.tensor.matmul(psSWs[q][pb:pb + 4, :], indS,
                                     Es[q][:, off:off + 512],
                                     start=True, stop=True,
                                     tile_position=(0, pb))
                    nc.tensor.matmul(psSWs[q][pb + 32:pb + 36, :], indW,
                                     Ps[q][:, off:off + 512],
                                     start=True, stop=True,
                                     tile_position=(0, pb + 32))
                    if i % 2 == 1:
                        del Es[q]
                        del Ps[q]

                # SC(t-8): one stats eviction per pair on ACT
                i = t - 8
                if 0 <= i < NB and i % 2 == 1:
                    q = i // 2
                    nc.scalar.copy(stats[:, q, :], psSWs[q][:])
                    del psSWs[q]

                # in-loop final reduction for ranges A and C, spread out
                # so no engine head-of-line blocks on a DMA completion
                if t == 25:   # SC(pair 7) issued at t=24
                    nc.sync.dma_start(
                        out=s128a[:, 0:64],
                        in_=stats[0:2, 0:8].rearrange("p a b -> p (a b)"))
                    nc.sync.dma_start(
                        out=s128a[:, 64:128],
                        in_=stats[64:66, 0:8].rearrange("p a b -> p (a b)"))
                    nc.sync.dma_start(
                        out=w128a[:, 0:64],
                        in_=stats[32:34, 0:8].rearrange("p a b -> p (a b)"))
                    nc.sync.dma_start(
                        out=w128a[:, 64:128],
                        in_=stats[96:98, 0:8].rearrange("p a b -> p (a b)"))
                elif t == 31:
                    nc.scalar.activation(lnA[:], s128a[:], Ln,
                                         accum_out=lsA[:])
                elif t == 33:
                    nc.scalar.activation(rSA[:], lnA[:], Exp, scale=-1.0)
                elif t == 35:
                    nc.vector.scalar_tensor_tensor(scrA[:], w128a[:], 1.0,
                                                   rSA[:], MUL, MUL,
                                                   accum_out=wsA[:])
                    nc.sync.dma_start(
                        out=s128c[:, 0:32],
                        in_=stats[0:2, 8:12].rearrange("p a b -> p (a b)"))
                    nc.sync.dma_start(
                        out=s128c[:, 32:64],
                        in_=stats[64:66, 8:12].rearrange("p a b -> p (a b)"))
                    nc.sync.dma_start(
                        out=w128c[:, 0:32],
                        in_=stats[32:34, 8:12].rearrange("p a b -> p (a b)"))
                    nc.sync.dma_start(
                        out=w128c[:, 32:64],
                        in_=stats[96:98, 8:12].rearrange("p a b -> p (a b)"))
                elif t == 39:
                    nc.scalar.activation(lnC[:], s128c[:], Ln,
                                         accum_out=lsC[:])
                elif t == 40:
                    nc.scalar.activation(rSC[:], lnC[:], Exp, scale=-1.0)

            # epilogue: finish range C, then range D = blocks [24:32]
            nc.vector.scalar_tensor_tensor(scrC[:], w128c[:], 1.0, rSC[:],
                                           MUL, MUL, accum_out=wsC[:])
            s128d = fin.tile([128, 64], F32)
            w128d = fin.tile([128, 64], F32)
            nc.sync.dma_start(
                out=s128d[:, 0:32],
                in_=stats[0:2, 12:16].rearrange("p a b -> p (a b)"))
            nc.sync.dma_start(
                out=s128d[:, 32:64],
                in_=stats[64:66, 12:16].rearrange("p a b -> p (a b)"))
            nc.sync.dma_start(
                out=w128d[:, 0:32],
                in_=stats[32:34, 12:16].rearrange("p a b -> p (a b)"))
            nc.sync.dma_start(
                out=w128d[:, 32:64],
                in_=stats[96:98, 12:16].rearrange("p a b -> p (a b)"))

            lnD = fin.tile([128, 64], F32)
            lsD = fin.tile([128, 1], F32)
            nc.scalar.activation(lnD[:], s128d[:], Ln, accum_out=lsD[:])
            rSD = fin.tile([128, 64], F32)
            nc.scalar.activation(rSD[:], lnD[:], Exp, scale=-1.0)
            scrD = fin.tile([128, 64], F32)
            wsD = fin.tile([128, 1], F32)
            nc.vector.scalar_tensor_tensor(scrD[:], w128d[:], 1.0, rSD[:],
                                           MUL, MUL, accum_out=wsD[:])
            lsAC = fin.tile([128, 1], F32)
            wsAC = fin.tile([128, 1], F32)
            nc.vector.tensor_tensor(lsAC[:], lsA[:], lsC[:], ADD)
            nc.vector.tensor_tensor(wsAC[:], wsA[:], wsC[:], ADD)
            ob = fin.tile([128, 2], F32)
            nc.vector.tensor_tensor(ob[:, 0:1], lsAC[:], lsD[:], ADD)
            nc.vector.tensor_tensor(ob[:, 1:2], wsAC[:], wsD[:], ADD)
            nc.sync.dma_start(out=out_ap[:], in_=ob[:])

    _CACHE["nc"] = nc
    _CACHE["octx"] = octx
    return nc


def _entropy_np(p):
    p = np.where(p <= 0, EPS, p)
    p = np.where(p >= 1, 1.0 - EPS, p)
    return -np.sum(p * np.log(p), axis=-1)


def kernel(x, m):
    nc = _build()

    mt2 = (2.0 * np.float64(m).T).astype(np.float32)       # [128, 64]
    consts = np.zeros((128, 200), dtype=ml_dtypes.bfloat16)
    consts[:, 0:64] = mt2.astype(ml_dtypes.bfloat16)       # pad64
    consts[0:K, 64] = 1                                    # indS col 0
    consts[K:128, 65] = 1                                  # indS col 1
    consts[0:K, 68] = 1                                    # indW col 0
    consts[K:128, 69] = 1                                  # indW col 1
    consts[:, 72:200] = np.eye(128, dtype=ml_dtypes.bfloat16)

    in_maps = []
    for c in range(NCORES):
        in_maps.append({
            "xs": np.ascontiguousarray(x[c * NSHARD:(c + 1) * NSHARD]),
            "consts": consts,
        })
    _CACHE["last_in_maps"] = in_maps
    res = run_bass_kernel_spmd(nc, in_maps, core_ids=list(range(NCORES)))

    tot_ls = 0.0
    tot_ws = 0.0
    for c in range(NCORES):
        o = np.float64(res.results[c]["out"])
        tot_ls += o[:, 0].sum()
        tot_ws += o[:, 1].sum()
    intra = (tot_ls - tot_ws) / N

    # inter term on host (tiny), replicating the reference exactly
    m64 = np.float64(m)
    mu = m64.mean(axis=0)
    d2 = ((mu[None, :] - m64) ** 2).sum(axis=1)
    zl = -d2
    zl -= zl.max()
    e = np.exp(zl)
    p = e / e.sum()
    inter = _entropy_np(p)

    total = intra - LAMB * inter
    return (np.float32(total), np.float32(intra), np.float32(inter))
